# revision 1
# baseline (speedup 1.0000x reference)
"""GNN IntraAgg kernel for Trainium2 (8 NeuronCores, SPMD data-parallel).

Computation (per node b):
    feats_1[b] = mean_k embedding[neighbor_idx[b, k]]      # [D]
    feats_2[b] = self_feats[b] - feats_1[b]                # [D]
    out[b]     = concat(feats_1[b], feats_2[b])            # [2D]

Sharding: batch axis split 8 ways (6250 nodes/core, padded to 6272 = 49*128);
embedding table replicated per core.

HW note: one indirect DMA consumes ONE offset per destination partition, so
each gather instruction fetches 128 table rows = 4 nodes x 32 neighbors
(slot-per-partition layout). The K-axis mean is a partition-axis reduction,
done on the TensorEngine: 32 accumulating matmuls per 128-node group against
constant 1/32 block-diagonal masks (lhsT[s, n] = 1/32 iff slot s belongs to
node n), leaving feats_1 for 128 nodes in one PSUM tile.

Host-side marshalling: neighbor_idx is transposed to [128, G*32] so that
column i of the SBUF index tile holds the 128 flat (node, k) slots of gather
instruction i; the masks are a compile-time constant shipped as an input.
"""

import numpy as np

N_EMBED, D = 200000, 128
B, K = 50000, 32
N_CORES = 8
P = 128
B_LOCAL = B // N_CORES            # 6250
G = (B_LOCAL + P - 1) // P        # 49 groups of 128 nodes
B_PAD = G * P                     # 6272
NPI = P // K                      # 4 nodes per gather instruction
JPG = P // NPI                    # 32 gather instructions per group
NI = G * JPG                      # 1568 gather instructions total

_cache: dict = {}


def make_masks() -> np.ndarray:
    """masks_t[s, j*128 + n] = 1/K iff n == 4*j + s//K  (lhsT layout)."""
    masks = np.zeros((JPG, P, P), np.float32)
    j = np.arange(JPG)[:, None]
    s = np.arange(P)[None, :]
    n = NPI * j + s // K                      # [JPG, P]
    masks[j, s, n] = 1.0 / K
    return np.ascontiguousarray(masks.transpose(1, 0, 2).reshape(P, JPG * P))


def build_bass(gather_bufs: int = 24):
    import concourse.bass as bass
    import concourse.mybir as mybir
    import concourse.tile as tile
    from concourse import bacc

    nc = bacc.Bacc(
        "TRN2",
        target_bir_lowering=False,
        debug=False,
        enable_asserts=True,
        num_devices=N_CORES,
    )
    emb = nc.dram_tensor(
        "embedding", [N_EMBED, D], mybir.dt.float32, kind="ExternalInput"
    ).ap()
    sf = nc.dram_tensor(
        "self_feats", [B_PAD, D], mybir.dt.float32, kind="ExternalInput"
    ).ap()
    nit = nc.dram_tensor(
        "neighbor_idx_t", [P, NI], mybir.dt.int32, kind="ExternalInput"
    ).ap()
    masks = nc.dram_tensor(
        "masks", [P, JPG * P], mybir.dt.float32, kind="ExternalInput"
    ).ap()
    out = nc.dram_tensor(
        "out", [B_PAD, 2 * D], mybir.dt.float32, kind="ExternalOutput"
    ).ap()

    with tile.TileContext(nc) as tc:
        with (
            tc.tile_pool(name="const", bufs=1) as const_tp,
            tc.tile_pool(name="gather", bufs=gather_bufs) as gather_tp,
            tc.tile_pool(name="psum", bufs=4, space="PSUM") as psum_tp,
            tc.tile_pool(name="io", bufs=6) as io_tp,
        ):
            idx_sb = const_tp.tile([P, NI], mybir.dt.int32, tag="idx")
            nc.sync.dma_start(out=idx_sb[:], in_=nit[:, :])
            mask_sb = const_tp.tile([P, JPG * P], mybir.dt.float32, tag="mask")
            nc.sync.dma_start(out=mask_sb[:], in_=masks[:, :])

            for g in range(G):
                r0 = g * P
                self_t = io_tp.tile([P, D], mybir.dt.float32, tag="self")
                nc.sync.dma_start(out=self_t[:], in_=sf[r0 : r0 + P, :])

                ps = psum_tp.tile([P, D], mybir.dt.float32, tag="ps")
                for j in range(JPG):
                    i = g * JPG + j
                    gt = gather_tp.tile([P, D], mybir.dt.float32, tag="g")
                    nc.gpsimd.indirect_dma_start(
                        out=gt[:],
                        out_offset=None,
                        in_=emb[:, :],
                        in_offset=bass.IndirectOffsetOnAxis(
                            ap=idx_sb[:, i : i + 1], axis=0
                        ),
                    )
                    nc.tensor.matmul(
                        out=ps[:],
                        lhsT=mask_sb[:, j * P : (j + 1) * P],
                        rhs=gt[:],
                        start=(j == 0),
                        stop=(j == JPG - 1),
                    )

                out_t = io_tp.tile([P, 2 * D], mybir.dt.float32, tag="out")
                nc.vector.tensor_copy(out=out_t[:, :D], in_=ps[:])
                nc.vector.tensor_tensor(
                    out=out_t[:, D:],
                    in0=self_t[:],
                    in1=ps[:],
                    op=mybir.AluOpType.subtract,
                )
                nc.sync.dma_start(out=out[r0 : r0 + P, :], in_=out_t[:])

    nc.compile()
    return nc


def make_in_maps(embedding, self_feats, neighbor_idx):
    embedding = np.ascontiguousarray(embedding, dtype=np.float32)
    sf = np.asarray(self_feats, dtype=np.float32).reshape(N_CORES, B_LOCAL, D)
    ni = np.asarray(neighbor_idx, dtype=np.int32).reshape(N_CORES, B_LOCAL, K)
    sf_pad = np.zeros((N_CORES, B_PAD, D), np.float32)
    ni_pad = np.zeros((N_CORES, B_PAD, K), np.int32)
    sf_pad[:, :B_LOCAL] = sf
    ni_pad[:, :B_LOCAL] = ni
    masks = make_masks()
    maps = []
    for c in range(N_CORES):
        # column i of neighbor_idx_t = flat (node, k) slots of instruction i
        nit = ni_pad[c].reshape(NI, P).T
        maps.append(
            {
                "embedding": embedding,
                "self_feats": np.ascontiguousarray(sf_pad[c]),
                "neighbor_idx_t": np.ascontiguousarray(nit),
                "masks": masks,
            }
        )
    return maps


def kernel(embedding, self_feats, neighbor_idx):
    from concourse import bass_utils

    if "nc" not in _cache:
        _cache["nc"] = build_bass()
    nc = _cache["nc"]
    in_maps = make_in_maps(embedding, self_feats, neighbor_idx)
    res = bass_utils.run_bass_kernel_spmd(nc, in_maps, core_ids=list(range(N_CORES)))
    outs = [res.results[c]["out"][:B_LOCAL] for c in range(N_CORES)]
    return np.concatenate(outs, axis=0)



# revision 3
# speedup vs baseline: 1.3630x; 1.3630x over previous
"""GNN IntraAgg kernel for Trainium2 (8 NeuronCores, SPMD data-parallel).

Computation (per node b):
    feats_1[b] = mean_k embedding[neighbor_idx[b, k]]      # [D]
    feats_2[b] = self_feats[b] - feats_1[b]                # [D]
    out[b]     = concat(feats_1[b], feats_2[b])            # [2D]

Sharding: batch axis split 8 ways (6250 nodes/core, padded to 6272 = 49*128);
each core receives a locality-partitioned copy of the embedding table.

Strategy. The gather is the whole problem (~1.6M random 512B rows). The only
TRN2 primitive that gathers at DMA line rate is InstDMAGatherAnt (dma_gather):
one instruction emits thousands of descriptors at ~0.34ns each, vs ~1.4us per
128 rows for generic indirect DMA (the previous kernel's bottleneck). Its
catch: int16 indices, so the source window is <=32768 rows.

Host marshalling therefore partitions each core's 49 node-groups into 7
chunks of 8 groups; one chunk's 32768 (node, k) references deduplicate to
~30.2k unique rows (50-sigma below the 32768 cap), which are packed into a
per-chunk window of an auxiliary table, and neighbor indices are rewritten to
window-local int16. The table is shipped as bf16, halving gather traffic
(512B -> 256B rows; tolerance is 2e-2, bf16 costs ~2e-3).

Each group is then ONE dma_gather of 4096 rows (single_packet=False -- the
single-packet path wedges above ~1k descriptors). Index layout puts gather
position j*128+p at partition p, column j, i.e. partition p holds node p's 32
neighbor rows contiguously, so the K-mean is a log-tree of 5 contiguous
tensor_tensor adds on the Vector engine (bf16 partials, fp32 final), the 1/K
scale rides the Scalar engine's activation-copy, and Vector does the
subtract. No TensorEngine, PSUM, or masks.
"""

import numpy as np
import ml_dtypes

N_EMBED, D = 200000, 128
B, K = 50000, 32
N_CORES = 8
P = 128
B_LOCAL = B // N_CORES            # 6250
G = (B_LOCAL + P - 1) // P        # 49 groups of 128 nodes
B_PAD = G * P                     # 6272
GPC = 8                           # groups per chunk
NCHUNK = (G + GPC - 1) // GPC     # 7
CH = 32768                        # chunk window rows (int16-addressable)
NI = P * K                        # 4096 gathered rows per group
C = NI // 16                      # idx columns per group (16-partition wrap)

_cache: dict = {}


def build_bass(gather_bufs: int = 4):
    import concourse.mybir as mybir
    import concourse.tile as tile
    from concourse import bacc, library_config

    nc = bacc.Bacc(
        "TRN2",
        target_bir_lowering=False,
        debug=False,
        enable_asserts=True,
        num_devices=N_CORES,
    )
    emb = nc.dram_tensor(
        "emb_aug", [NCHUNK * CH, D], mybir.dt.bfloat16, kind="ExternalInput"
    ).ap()
    sf = nc.dram_tensor(
        "self_feats", [B_PAD, D], mybir.dt.float32, kind="ExternalInput"
    ).ap()
    nit = nc.dram_tensor(
        "neighbor_idx_t", [P, G * C], mybir.dt.int16, kind="ExternalInput"
    ).ap()
    out = nc.dram_tensor(
        "out", [B_PAD, 2 * D], mybir.dt.float32, kind="ExternalOutput"
    ).ap()

    with (
        tile.TileContext(nc) as tc,
        tc.tile_pool(name="const", bufs=1) as const_tp,
        tc.tile_pool(name="gather", bufs=gather_bufs) as gather_tp,
        tc.tile_pool(name="tree", bufs=3) as tree_tp,
        tc.tile_pool(name="io", bufs=6) as io_tp,
    ):
        nc.gpsimd.load_library(library_config.mlp)
        idx_sb = const_tp.tile([P, G * C], mybir.dt.int16, tag="idx")
        nc.sync.dma_start(out=idx_sb[:], in_=nit[:, :])

        for g in range(G):
            r0 = g * P
            chunk = g // GPC
            self_t = io_tp.tile([P, D], mybir.dt.float32, tag="self")
            nc.sync.dma_start(out=self_t[:], in_=sf[r0 : r0 + P, :])

            gt = gather_tp.tile([P, K * D], mybir.dt.bfloat16, tag="g")
            nc.gpsimd.dma_gather(
                out_ap=gt[:].rearrange("p (c e) -> p c e", e=D),
                in_ap=emb[chunk * CH : (chunk + 1) * CH, :],
                idxs_ap=idx_sb[:, g * C : (g + 1) * C],
                num_idxs=NI,
                num_idxs_reg=NI,
                elem_size=D,
                single_packet=False,
            )

            # K-mean as a contiguous halving tree: 32 -> 16 -> 8 -> 4 -> 2 -> 1
            t16 = tree_tp.tile([P, 16 * D], mybir.dt.bfloat16, tag="t16")
            nc.vector.tensor_tensor(
                out=t16[:], in0=gt[:, : 16 * D], in1=gt[:, 16 * D :],
                op=mybir.AluOpType.add,
            )
            t8 = tree_tp.tile([P, 8 * D], mybir.dt.bfloat16, tag="t8")
            nc.vector.tensor_tensor(
                out=t8[:], in0=t16[:, : 8 * D], in1=t16[:, 8 * D :],
                op=mybir.AluOpType.add,
            )
            t4 = tree_tp.tile([P, 4 * D], mybir.dt.bfloat16, tag="t4")
            nc.vector.tensor_tensor(
                out=t4[:], in0=t8[:, : 4 * D], in1=t8[:, 4 * D :],
                op=mybir.AluOpType.add,
            )
            t2 = tree_tp.tile([P, 2 * D], mybir.dt.bfloat16, tag="t2")
            nc.vector.tensor_tensor(
                out=t2[:], in0=t4[:, : 2 * D], in1=t4[:, 2 * D :],
                op=mybir.AluOpType.add,
            )
            t1 = tree_tp.tile([P, D], mybir.dt.float32, tag="t1")
            nc.vector.tensor_tensor(
                out=t1[:], in0=t2[:, :D], in1=t2[:, D:],
                op=mybir.AluOpType.add,
            )

            out_t = io_tp.tile([P, 2 * D], mybir.dt.float32, tag="out")
            nc.scalar.activation(
                out=out_t[:, :D], in_=t1[:],
                func=mybir.ActivationFunctionType.Copy, scale=1.0 / K,
            )
            nc.vector.tensor_tensor(
                out=out_t[:, D:], in0=self_t[:], in1=out_t[:, :D],
                op=mybir.AluOpType.subtract,
            )
            nc.sync.dma_start(out=out[r0 : r0 + P, :], in_=out_t[:])

    nc.compile()
    return nc


def make_in_maps(embedding, self_feats, neighbor_idx):
    emb_bf = np.asarray(embedding, np.float32).astype(ml_dtypes.bfloat16)
    sf = np.asarray(self_feats, dtype=np.float32).reshape(N_CORES, B_LOCAL, D)
    ni = np.asarray(neighbor_idx, dtype=np.int64).reshape(N_CORES, B_LOCAL, K)
    sf_pad = np.zeros((N_CORES, B_PAD, D), np.float32)
    ni_pad = np.zeros((N_CORES, B_PAD, K), np.int64)
    sf_pad[:, :B_LOCAL] = sf
    ni_pad[:, :B_LOCAL] = ni
    maps = []
    for c in range(N_CORES):
        nip = ni_pad[c]
        emb_aug = np.zeros((NCHUNK * CH, D), ml_dtypes.bfloat16)
        idx_t = np.zeros((P, G * C), np.int16)
        for q in range(NCHUNK):
            g0, g1 = GPC * q, min(GPC * q + GPC, G)
            sl = nip[g0 * P : g1 * P]               # [(g1-g0)*P, K]
            rows = np.unique(sl)                    # sorted unique global rows
            assert len(rows) <= CH, f"chunk {q} overflow: {len(rows)}"
            emb_aug[q * CH : q * CH + len(rows)] = emb_bf[rows]
            loc = np.searchsorted(rows, sl).astype(np.int16)
            for g in range(g0, g1):
                lg = loc[(g - g0) * P : (g - g0 + 1) * P]   # [P, K]
                flat = lg.T.ravel()                 # position j*128+p
                block = flat.reshape(C, 16).T       # [16, C]
                idx_t[:, g * C : (g + 1) * C] = np.tile(block, (8, 1))
        maps.append(
            {
                "emb_aug": emb_aug,
                "self_feats": np.ascontiguousarray(sf_pad[c]),
                "neighbor_idx_t": idx_t,
            }
        )
    return maps


def kernel(embedding, self_feats, neighbor_idx):
    from concourse import bass_utils

    if "nc" not in _cache:
        _cache["nc"] = build_bass()
    nc = _cache["nc"]
    in_maps = make_in_maps(embedding, self_feats, neighbor_idx)
    res = bass_utils.run_bass_kernel_spmd(nc, in_maps, core_ids=list(range(N_CORES)))
    outs = [res.results[c]["out"][:B_LOCAL] for c in range(N_CORES)]
    return np.concatenate(outs, axis=0)


# revision 5
# speedup vs baseline: 4.7616x; 3.4933x over previous
"""GNN IntraAgg kernel for Trainium2 (8 NeuronCores, SPMD data-parallel).

Computation (per node b):
    feats_1[b] = mean_k embedding[neighbor_idx[b, k]]      # [D]
    feats_2[b] = self_feats[b] - feats_1[b]                # [D]
    out[b]     = concat(feats_1[b], feats_2[b])            # [2D]

Sharding: batch axis split 8 ways (6250 nodes/core, padded to 6272 = 49*128);
each core receives a locality-partitioned copy of the embedding table.

Strategy. The gather is the whole problem (~1.6M random 512B rows). The only
TRN2 primitive that gathers at DMA line rate is InstDMAGatherAnt (dma_gather):
one instruction emits thousands of descriptors at ~0.34ns each, vs ~1.4us per
128 rows for generic indirect DMA (the previous kernel's bottleneck). Its
catch: int16 indices, so the source window is <=32768 rows.

Host marshalling therefore partitions each core's 49 node-groups into 7
chunks of 8 groups; one chunk's 32768 (node, k) references deduplicate to
~30.2k unique rows (50-sigma below the 32768 cap), which are packed into a
per-chunk window of an auxiliary table, and neighbor indices are rewritten to
window-local int16. The table is shipped as bf16, halving gather traffic
(512B -> 256B rows; tolerance is 2e-2, bf16 costs ~2e-3).

Each group is then ONE dma_gather of 4096 rows (single_packet=False -- the
single-packet path wedges above ~1k descriptors). Index layout puts gather
position j*128+p at partition p, column j, i.e. partition p holds node p's 32
neighbor rows contiguously, so the K-mean is a log-tree of 5 contiguous
tensor_tensor adds on the Vector engine (bf16 partials, fp32 final), the 1/K
scale rides the Scalar engine's activation-copy, and Vector does the
subtract. No TensorEngine, PSUM, or masks.
"""

import numpy as np
import ml_dtypes

N_EMBED, D = 200000, 128
B, K = 50000, 32
N_CORES = 8
P = 128
B_LOCAL = B // N_CORES            # 6250
G = (B_LOCAL + P - 1) // P        # 49 groups of 128 nodes
B_PAD = G * P                     # 6272
GPC = 8                           # groups per chunk
NCHUNK = (G + GPC - 1) // GPC     # 7
CH = 32768                        # chunk window rows (int16-addressable)
NI = P * K                        # 4096 gathered rows per group
C = NI // 16                      # idx columns per group (16-partition wrap)

_cache: dict = {}


def build_bass(gather_bufs: int = 8, n_queues: int = 4):
    import concourse.mybir as mybir
    import concourse.tile as tile
    from concourse import bacc, library_config

    nc = bacc.Bacc(
        "TRN2",
        target_bir_lowering=False,
        debug=False,
        enable_asserts=True,
        num_devices=N_CORES,
        num_swdge_queues=n_queues,
    )
    emb = nc.dram_tensor(
        "emb_aug", [NCHUNK * CH, D], mybir.dt.bfloat16, kind="ExternalInput"
    ).ap()
    sf = nc.dram_tensor(
        "self_feats", [B_PAD, D], mybir.dt.float32, kind="ExternalInput"
    ).ap()
    nit = nc.dram_tensor(
        "neighbor_idx_t", [P, G * C], mybir.dt.int16, kind="ExternalInput"
    ).ap()
    out = nc.dram_tensor(
        "out", [B_PAD, 2 * D], mybir.dt.float32, kind="ExternalOutput"
    ).ap()

    with (
        tile.TileContext(nc) as tc,
        tc.tile_pool(name="const", bufs=1) as const_tp,
        tc.tile_pool(name="gather", bufs=gather_bufs) as gather_tp,
        tc.tile_pool(name="tree", bufs=3) as tree_tp,
        tc.tile_pool(name="io", bufs=6) as io_tp,
    ):
        nc.gpsimd.load_library(library_config.mlp)
        idx_sb = const_tp.tile([P, G * C], mybir.dt.int16, tag="idx")
        nc.sync.dma_start(out=idx_sb[:], in_=nit[:, :])

        for g in range(G):
            r0 = g * P
            chunk = g // GPC
            self_t = io_tp.tile([P, D], mybir.dt.float32, tag="self")
            nc.sync.dma_start(out=self_t[:], in_=sf[r0 : r0 + P, :])

            gt = gather_tp.tile([P, K * D], mybir.dt.bfloat16, tag="g")
            nc.gpsimd.dma_gather(
                out_ap=gt[:].rearrange("p (c e) -> p c e", e=D),
                in_ap=emb[chunk * CH : (chunk + 1) * CH, :],
                idxs_ap=idx_sb[:, g * C : (g + 1) * C],
                num_idxs=NI,
                num_idxs_reg=NI,
                elem_size=D,
                single_packet=False,
                queue_num=g % n_queues,
            )

            # K-mean as a contiguous halving tree: 32 -> 16 -> 8 -> 4 -> 2 -> 1
            t16 = tree_tp.tile([P, 16 * D], mybir.dt.bfloat16, tag="t16")
            nc.vector.tensor_tensor(
                out=t16[:], in0=gt[:, : 16 * D], in1=gt[:, 16 * D :],
                op=mybir.AluOpType.add,
            )
            t8 = tree_tp.tile([P, 8 * D], mybir.dt.bfloat16, tag="t8")
            nc.vector.tensor_tensor(
                out=t8[:], in0=t16[:, : 8 * D], in1=t16[:, 8 * D :],
                op=mybir.AluOpType.add,
            )
            t4 = tree_tp.tile([P, 4 * D], mybir.dt.bfloat16, tag="t4")
            nc.vector.tensor_tensor(
                out=t4[:], in0=t8[:, : 4 * D], in1=t8[:, 4 * D :],
                op=mybir.AluOpType.add,
            )
            t2 = tree_tp.tile([P, 2 * D], mybir.dt.bfloat16, tag="t2")
            nc.vector.tensor_tensor(
                out=t2[:], in0=t4[:, : 2 * D], in1=t4[:, 2 * D :],
                op=mybir.AluOpType.add,
            )
            t1 = tree_tp.tile([P, D], mybir.dt.float32, tag="t1")
            nc.vector.tensor_tensor(
                out=t1[:], in0=t2[:, :D], in1=t2[:, D:],
                op=mybir.AluOpType.add,
            )

            out_t = io_tp.tile([P, 2 * D], mybir.dt.float32, tag="out")
            nc.scalar.activation(
                out=out_t[:, :D], in_=t1[:],
                func=mybir.ActivationFunctionType.Copy, scale=1.0 / K,
            )
            nc.vector.tensor_tensor(
                out=out_t[:, D:], in0=self_t[:], in1=out_t[:, :D],
                op=mybir.AluOpType.subtract,
            )
            nc.sync.dma_start(out=out[r0 : r0 + P, :], in_=out_t[:])

    nc.compile()
    return nc


def make_in_maps(embedding, self_feats, neighbor_idx):
    emb_bf = np.asarray(embedding, np.float32).astype(ml_dtypes.bfloat16)
    sf = np.asarray(self_feats, dtype=np.float32).reshape(N_CORES, B_LOCAL, D)
    ni = np.asarray(neighbor_idx, dtype=np.int64).reshape(N_CORES, B_LOCAL, K)
    sf_pad = np.zeros((N_CORES, B_PAD, D), np.float32)
    ni_pad = np.zeros((N_CORES, B_PAD, K), np.int64)
    sf_pad[:, :B_LOCAL] = sf
    ni_pad[:, :B_LOCAL] = ni
    maps = []
    for c in range(N_CORES):
        nip = ni_pad[c]
        emb_aug = np.zeros((NCHUNK * CH, D), ml_dtypes.bfloat16)
        idx_t = np.zeros((P, G * C), np.int16)
        for q in range(NCHUNK):
            g0, g1 = GPC * q, min(GPC * q + GPC, G)
            sl = nip[g0 * P : g1 * P]               # [(g1-g0)*P, K]
            rows = np.unique(sl)                    # sorted unique global rows
            assert len(rows) <= CH, f"chunk {q} overflow: {len(rows)}"
            emb_aug[q * CH : q * CH + len(rows)] = emb_bf[rows]
            loc = np.searchsorted(rows, sl).astype(np.int16)
            for g in range(g0, g1):
                lg = loc[(g - g0) * P : (g - g0 + 1) * P]   # [P, K]
                flat = lg.T.ravel()                 # position j*128+p
                block = flat.reshape(C, 16).T       # [16, C]
                idx_t[:, g * C : (g + 1) * C] = np.tile(block, (8, 1))
        maps.append(
            {
                "emb_aug": emb_aug,
                "self_feats": np.ascontiguousarray(sf_pad[c]),
                "neighbor_idx_t": idx_t,
            }
        )
    return maps


def kernel(embedding, self_feats, neighbor_idx):
    from concourse import bass_utils

    if "nc" not in _cache:
        _cache["nc"] = build_bass()
    nc = _cache["nc"]
    in_maps = make_in_maps(embedding, self_feats, neighbor_idx)
    res = bass_utils.run_bass_kernel_spmd(nc, in_maps, core_ids=list(range(N_CORES)))
    outs = [res.results[c]["out"][:B_LOCAL] for c in range(N_CORES)]
    return np.concatenate(outs, axis=0)
